# revision 22
# baseline (speedup 1.0000x reference)
"""AvgPool2d(64x64, stride 1) with replicate-padding back to (512, 512),
as a distributed Bass kernel on 8 TRN2 NeuronCores.

Input : x (8, 64, 512, 512) float32
Output: (8, 64, 512, 512) float32

Strategy (pure data parallel): one batch element per core. Per core the
pooling is a separable 64-wide box filter; both directions run on the
TensorEngine as matmuls against a banded 0/1-matrix `band` [512, 512]
with band[h, i] = 1/64 iff clamp(i-31, 0, 448) <= h < clamp(...) + 64
(the clamp folds the replicate-padding, the 1/64 folds the averaging).

    V^T = (X^T @ band)        pass 1: vertical box mean, transposed
    O   = (V^T)^T @ band      pass 2: horizontal box mean, natural

Both passes keep the *data* tile in the stationary (lhsT) operand and
the band in the moving operand, which avoids every transpose.

V5 (memory roofline, ~170 us HBM floor):
- x/out live in DRAM as bf16 (host casts on upload / upcasts on
  download; kernel always computed in bf16) -> 60.5 MB/core traffic.
- SDMA engines pay ~20 ns per descriptor, so the host stores both x and
  out PRE-BLOCKED per channel as [C, 128, rows, 512]: partition p's
  slice of a channel is one contiguous 4KB chunk, while SBUF receives
  the standard 128-row block layout (h = 128k + p) that gives
  minimal-width matmul plans. Per-channel transfers keep dependency
  granularity fine (deep prefetch, short drain tail).
- Input loads ride the sync/SP HWDGE ring; output stores are dispatched
  from the otherwise-idle GpSimd (SWDGE) queue so no compute engine's
  instruction stream ever blocks on a store's copy dependencies.
- Only output rows 28..483 are computed/stored (456 = 114 x 4;
  the rest is edge replication the host reapplies), and pass 1 only
  produces i in [28, 484). vtb stays 512 columns (pad junk) so pass-2
  lhsT slices are 128-wide: narrower weights disable FWL and double
  LDWEIGHTS time.
- PSUM->SBUF copies are the scarcest engine resource (~1.6 ns/free-elem
  on Vector AND Scalar): PSUM tiles are 2-bank pairs (each matmul
  region bank-aligned) so each copy moves FD~1024 in one instruction,
  and copies are greedily load-balanced between Vector and Scalar.
- Software pipelining: pass 1 of channel c+1 is emitted before pass 2
  of channel c, so the PE streams through copy latencies.
"""

import numpy as np
import ml_dtypes

C, H, W = 64, 512, 512
P = 128
NKH = H // P  # 4 partition blocks
KERNEL = 64
OUT_VALID = H - KERNEL + 1  # 449
PT = (H - OUT_VALID) // 2  # 31 (left/top pad)
G = 4  # channels per output store group (16KB descriptors)
NG = C // G
OLO, OHI = 28, 484  # computed output-row range (quad-aligned cover of valid)
NOUT = OHI - OLO  # 456
NP_OUT = NOUT // 4  # 114 output partitions

# Matmul plan for one PSUM tile, contraction over standard 128-row
# blocks k; each instruction's column range is uniformly "first writer"
# or "accumulating" so per-element PSUM has_written semantics hold:
# (k, lo, hi, start, stop).
MM_PLAN_BLOCK = [
    (0, 0, 159, True, False),
    (1, 96, 159, False, False),
    (1, 159, 287, False, False),
    (2, 224, 287, False, False),
    (2, 287, 415, False, False),
    (3, 352, 415, False, False),
    (3, 415, 512, False, True),
]
# Pass-1 plan: same structure, i-range clipped to [OLO, OHI).
MM_PLAN_TRIM = [
    (k, max(lo, OLO), min(hi, OHI), start, stop)
    for (k, lo, hi, start, stop) in MM_PLAN_BLOCK
]


def make_band() -> np.ndarray:
    i = np.arange(H)
    ic = np.clip(i - PT, 0, OUT_VALID - 1)
    h = np.arange(H)
    band = (h[:, None] >= ic[None, :]) & (h[:, None] < ic[None, :] + KERNEL)
    return (band.astype(np.float32) / KERNEL).astype(ml_dtypes.bfloat16)


class CopyBalancer:
    """Greedy build-time assignment of PSUM->SBUF copies to the two
    engines with PSUM ports, weighted by their measured per-copy cost."""

    def __init__(self, nc):
        self.nc = nc
        self.load = {"scalar": 0.0, "vector": 0.0}

    def copy(self, dst, src, fd):
        cost = {"scalar": 172 + fd / 1.2, "vector": (120 + fd) / 0.96}
        eng = min(cost, key=lambda e: self.load[e] + cost[e])
        self.load[eng] += cost[eng]
        if eng == "scalar":
            self.nc.scalar.copy(dst, src)
        else:
            self.nc.vector.tensor_copy(dst, src)

    def copy_fixed(self, dst, src, which):
        if which == 0:
            self.nc.scalar.copy(dst, src)
        else:
            self.nc.vector.tensor_copy(dst, src)


def build_avgpool(tc, x_ap, band_ap, out_ap):
    import concourse.mybir as mybir

    nc = tc.nc
    f32 = mybir.dt.float32
    bf16 = mybir.dt.bfloat16
    cb = CopyBalancer(nc)

    with (
        tc.tile_pool(name="const", bufs=1) as const_pool,
        tc.tile_pool(name="xin", bufs=8) as xin_pool,
        tc.tile_pool(name="vt", bufs=3) as vt_pool,
        tc.tile_pool(name="oout", bufs=4) as out_pool,
        tc.tile_pool(name="vtps", bufs=2, space="PSUM") as vt_psum,
        tc.tile_pool(name="ops", bufs=2, space="PSUM") as o_psum,
    ):
        # band in standard block layout: [p, k, i] = band[128*k + p, i]
        band_t = const_pool.tile([P, NKH, H], bf16, tag="band")
        nc.sync.dma_start(band_t[:], band_ap.rearrange("(kh p) i -> p kh i", p=P))

        vtbs = {}

        def pass1(c):
            xb = xin_pool.tile([P, NKH, W], bf16, tag="xb")
            nc.sync.dma_start(xb[:], x_ap[c])
            # padded to 512 cols: pass-2 lhsT slices t::4 must be
            # 128-wide or FWL turns off (needs NumWeights==128) and
            # LDWEIGHTS runs 2x slow; pad cols feed only PSUM partitions
            # >= 114, which are never copied out
            vtb = vt_pool.tile([P, NKH, W], bf16, tag="vtb")
            vtbs[c] = vtb
            for half in range(2):
                # each m-slice is bank-aligned: a matmul output region
                # must not straddle a 2KB PSUM bank boundary
                vt_ps = vt_psum.tile([P, 2, W], f32, tag="vt")
                for m in range(2):
                    mw = 2 * half + m
                    for k, lo, hi, start, stop in MM_PLAN_TRIM:
                        nc.tensor.matmul(
                            vt_ps[:, m, lo - OLO : hi - OLO],
                            xb[:, k, P * mw : P * (mw + 1)],
                            band_t[:, k, lo:hi],
                            start=start,
                            stop=stop,
                        )
                # the two halves go to different engines so pass 2's
                # wait on both copies is one copy-latency, not two;
                # alternating per channel keeps long-run load balanced
                cb.copy_fixed(
                    vtb[:, 2 * half : 2 * half + 2, :NOUT],
                    vt_ps[:, :, :NOUT],
                    (c + half) % 2,
                )

        osbs = {}

        def pass2(c):
            g, ci = divmod(c, G)
            if ci == 0:
                o_sb_new = out_pool.tile([NP_OUT, G, 4, W], bf16, tag="osb")
                osbs[g] = o_sb_new
            o_sb = osbs[g]
            vtb = vtbs.pop(c)
            for half in range(2):
                o_ps = o_psum.tile([P, 2, W], f32, tag="o")
                for m in range(2):
                    t = 2 * half + m
                    for k, lo, hi, start, stop in MM_PLAN_BLOCK:
                        nc.tensor.matmul(
                            o_ps[:, m, lo:hi],
                            vtb[:, k, t : W : 4],
                            band_t[:, k, lo:hi],
                            start=start,
                            stop=stop,
                        )
                cb.copy_fixed(
                    o_sb[:, ci, 2 * half : 2 * half + 2, :],
                    o_ps[:NP_OUT],
                    (c + half) % 2,
                )
            if ci == G - 1:
                # one 16KB-per-partition store per 4-channel group,
                # dispatched from the otherwise-idle GpSimd (SWDGE)
                # queue (a HWDGE dispatch would block the sync/scalar
                # instruction stream on this group's copy dependencies,
                # starving input prefetch / later copies)
                nc.gpsimd.dma_start(out_ap[g], osbs.pop(g)[:])

        # software pipeline: PE runs pass1(c+1) while pass2(c) waits on
        # pass-1 copies
        pass1(0)
        for c in range(1, C):
            pass1(c)
            pass2(c - 1)
        pass2(C - 1)


def build_nc():
    import concourse.mybir as mybir
    import concourse.tile as tile
    from concourse import bacc

    # Bacc (not raw Bass): its compile() runs generate_event_semaphores,
    # which splits multi-semaphore waits — walrus codegen allows at most
    # one wait command per DMA instruction.
    nc = bacc.Bacc()
    x = nc.dram_tensor(
        "x", [C, P, NKH, W], mybir.dt.bfloat16, kind="ExternalInput"
    )
    band = nc.dram_tensor("band", [H, W], mybir.dt.bfloat16, kind="ExternalInput")
    out = nc.dram_tensor(
        "out", [NG, NP_OUT, G, 4, W], mybir.dt.bfloat16, kind="ExternalOutput"
    )
    with tile.TileContext(nc) as tc:
        build_avgpool(tc, x.ap(), band.ap(), out.ap())
    nc.compile()
    return nc


def _ensure_axon_ntff_hook():
    """If tracing is requested (BASS_TRACE) under axon, run_bass_kernel_spmd
    imports antenv.axon_hooks, which some agent images lack. Install the
    real hook if possible, else a stub that degrades tracing gracefully."""
    import sys
    import types

    try:
        import antenv.axon_hooks  # noqa: F401

        return
    except Exception:
        pass
    try:
        import antenv
    except Exception:
        return
    mod = types.ModuleType("antenv.axon_hooks")
    mod._hook = None
    mod.set_axon_ntff_profile_hook = lambda h: setattr(mod, "_hook", h)
    mod.get_axon_ntff_profile_hook = lambda: mod._hook
    sys.modules["antenv.axon_hooks"] = mod
    antenv.axon_hooks = mod
    try:
        from trn_agent_boot.trn_boot import _ntff_profile_via_ctypes

        hook = _ntff_profile_via_ctypes("/opt/axon/libaxon_pjrt.so")
        if hook is not None:
            mod.set_axon_ntff_profile_hook(hook)
    except Exception:
        pass


def prep_inputs(x: np.ndarray):
    """Shard, cast, and pre-block the full input for the 8 cores.

    Device x layout: x_dev[c, p, k, w] = x[c, 128k+p, w]
    """
    x = np.asarray(x, dtype=np.float32)
    assert x.shape == (8, C, H, W)
    xb = x.astype(ml_dtypes.bfloat16)
    band = make_band()
    in_maps = []
    for b in range(8):
        xd = np.ascontiguousarray(xb[b].reshape(C, NKH, P, W).transpose(0, 2, 1, 3))
        in_maps.append({"x": xd, "band": band})
    return in_maps


def gather_output(results) -> np.ndarray:
    """Unshard, un-block, upcast, and re-apply the replicate padding.

    Device out layout: out_dev[g, p, ci, t, w] = out[4g+ci, OLO+4p+t, w]
    """
    full = np.empty((8, C, H, W), dtype=np.float32)
    for b, r in enumerate(results):
        od = np.asarray(r["out"])  # [NG, NP_OUT, G, 4, W] bf16
        rows = od.transpose(0, 2, 1, 3, 4).reshape(C, NOUT, W).astype(np.float32)
        full[b, :, OLO:OHI] = rows
        full[b, :, :OLO] = rows[:, PT - OLO : PT - OLO + 1]
        full[b, :, OHI:] = rows[:, OUT_VALID + PT - 1 - OLO : OUT_VALID + PT - OLO]
    return full


def kernel(x) -> np.ndarray:
    _ensure_axon_ntff_hook()
    from concourse.bass_utils import run_bass_kernel_spmd

    nc = build_nc()
    in_maps = prep_inputs(x)
    res = run_bass_kernel_spmd(nc, in_maps, core_ids=list(range(8)))
    return gather_output(res.results)


# revision 23
# speedup vs baseline: 1.0507x; 1.0507x over previous
"""AvgPool2d(64x64, stride 1) with replicate-padding back to (512, 512),
as a distributed Bass kernel on 8 TRN2 NeuronCores.

Input : x (8, 64, 512, 512) float32
Output: (8, 64, 512, 512) float32

Strategy (pure data parallel): one batch element per core. Per core the
pooling is a separable 64-wide box filter; both directions run on the
TensorEngine as matmuls against a banded 0/1-matrix `band` [512, 512]
with band[h, i] = 1/64 iff clamp(i-31, 0, 448) <= h < clamp(...) + 64
(the clamp folds the replicate-padding, the 1/64 folds the averaging).

    V^T = (X^T @ band)        pass 1: vertical box mean, transposed
    O   = (V^T)^T @ band      pass 2: horizontal box mean, natural

Both passes keep the *data* tile in the stationary (lhsT) operand and
the band in the moving operand, which avoids every transpose.

V5 (memory roofline, ~170 us HBM floor):
- x/out live in DRAM as bf16 (host casts on upload / upcasts on
  download; kernel always computed in bf16) -> 60.5 MB/core traffic.
- SDMA engines pay ~20 ns per descriptor, so the host stores both x and
  out PRE-BLOCKED per channel as [C, 128, rows, 512]: partition p's
  slice of a channel is one contiguous 4KB chunk, while SBUF receives
  the standard 128-row block layout (h = 128k + p) that gives
  minimal-width matmul plans. Per-channel transfers keep dependency
  granularity fine (deep prefetch, short drain tail).
- Input loads ride the sync/SP HWDGE ring; output stores are dispatched
  from the otherwise-idle GpSimd (SWDGE) queue so no compute engine's
  instruction stream ever blocks on a store's copy dependencies.
- Only output rows 28..483 are computed/stored (456 = 114 x 4;
  the rest is edge replication the host reapplies), and pass 1 only
  produces i in [28, 484). vtb stays 512 columns (pad junk) so pass-2
  lhsT slices are 128-wide: narrower weights disable FWL and double
  LDWEIGHTS time.
- PSUM->SBUF copies are the scarcest engine resource (~1.6 ns/free-elem
  on Vector AND Scalar): PSUM tiles are 2-bank pairs (each matmul
  region bank-aligned) so each copy moves FD~1024 in one instruction,
  and copies are greedily load-balanced between Vector and Scalar.
- Software pipelining: pass 1 of channel c+1 is emitted before pass 2
  of channel c, so the PE streams through copy latencies.
"""

import numpy as np
import ml_dtypes

C, H, W = 64, 512, 512
P = 128
NKH = H // P  # 4 partition blocks
KERNEL = 64
OUT_VALID = H - KERNEL + 1  # 449
PT = (H - OUT_VALID) // 2  # 31 (left/top pad)
G = 4  # channels per output store group (16KB descriptors)
NG = C // G
OLO, OHI = 28, 484  # computed output-row range (quad-aligned cover of valid)
NOUT = OHI - OLO  # 456
NP_OUT = NOUT // 4  # 114 output partitions

# Matmul plan for one PSUM tile, contraction over standard 128-row
# blocks k; each instruction's column range is uniformly "first writer"
# or "accumulating" so per-element PSUM has_written semantics hold:
# (k, lo, hi, start, stop).
MM_PLAN_BLOCK = [
    (0, 0, 159, True, False),
    (1, 96, 159, False, False),
    (1, 159, 287, False, False),
    (2, 224, 287, False, False),
    (2, 287, 415, False, False),
    (3, 352, 415, False, False),
    (3, 415, 512, False, True),
]
# Pass-1 plan: same structure, i-range clipped to [OLO, OHI).
MM_PLAN_TRIM = [
    (k, max(lo, OLO), min(hi, OHI), start, stop)
    for (k, lo, hi, start, stop) in MM_PLAN_BLOCK
]


def make_band() -> np.ndarray:
    i = np.arange(H)
    ic = np.clip(i - PT, 0, OUT_VALID - 1)
    h = np.arange(H)
    band = (h[:, None] >= ic[None, :]) & (h[:, None] < ic[None, :] + KERNEL)
    return (band.astype(np.float32) / KERNEL).astype(ml_dtypes.bfloat16)


class CopyBalancer:
    """Greedy build-time assignment of PSUM->SBUF copies to the two
    engines with PSUM ports, weighted by their measured per-copy cost."""

    def __init__(self, nc):
        self.nc = nc
        self.load = {"scalar": 0.0, "vector": 0.0}

    def copy(self, dst, src, fd):
        cost = {"scalar": 172 + fd / 1.2, "vector": (120 + fd) / 0.96}
        eng = min(cost, key=lambda e: self.load[e] + cost[e])
        self.load[eng] += cost[eng]
        if eng == "scalar":
            self.nc.scalar.copy(dst, src)
        else:
            self.nc.vector.tensor_copy(dst, src)



def build_avgpool(tc, x_ap, band_ap, out_ap):
    import concourse.mybir as mybir

    nc = tc.nc
    f32 = mybir.dt.float32
    bf16 = mybir.dt.bfloat16
    cb = CopyBalancer(nc)

    with (
        tc.tile_pool(name="const", bufs=1) as const_pool,
        tc.tile_pool(name="xin", bufs=8) as xin_pool,
        tc.tile_pool(name="vt", bufs=3) as vt_pool,
        tc.tile_pool(name="oout", bufs=4) as out_pool,
        tc.tile_pool(name="vtps", bufs=2, space="PSUM") as vt_psum,
        tc.tile_pool(name="ops", bufs=2, space="PSUM") as o_psum,
    ):
        # band in standard block layout: [p, k, i] = band[128*k + p, i]
        band_t = const_pool.tile([P, NKH, H], bf16, tag="band")
        nc.sync.dma_start(band_t[:], band_ap.rearrange("(kh p) i -> p kh i", p=P))

        vtbs = {}

        def pass1(c):
            xb = xin_pool.tile([P, NKH, W], bf16, tag="xb")
            nc.sync.dma_start(xb[:], x_ap[c])
            # padded to 512 cols: pass-2 lhsT slices t::4 must be
            # 128-wide or FWL turns off (needs NumWeights==128) and
            # LDWEIGHTS runs 2x slow; pad cols feed only PSUM partitions
            # >= 114, which are never copied out
            vtb = vt_pool.tile([P, NKH, W], bf16, tag="vtb")
            vtbs[c] = vtb
            for half in range(2):
                # each m-slice is bank-aligned: a matmul output region
                # must not straddle a 2KB PSUM bank boundary
                vt_ps = vt_psum.tile([P, 2, W], f32, tag="vt")
                for m in range(2):
                    mw = 2 * half + m
                    for k, lo, hi, start, stop in MM_PLAN_TRIM:
                        nc.tensor.matmul(
                            vt_ps[:, m, lo - OLO : hi - OLO],
                            xb[:, k, P * mw : P * (mw + 1)],
                            band_t[:, k, lo:hi],
                            start=start,
                            stop=stop,
                        )
                cb.copy(
                    vtb[:, 2 * half : 2 * half + 2, :NOUT],
                    vt_ps[:, :, :NOUT],
                    2 * NOUT,
                )

        osbs = {}

        def pass2(c):
            g, ci = divmod(c, G)
            if ci == 0:
                o_sb_new = out_pool.tile([NP_OUT, G, 4, W], bf16, tag="osb")
                osbs[g] = o_sb_new
            o_sb = osbs[g]
            vtb = vtbs.pop(c)
            for half in range(2):
                o_ps = o_psum.tile([P, 2, W], f32, tag="o")
                for m in range(2):
                    t = 2 * half + m
                    for k, lo, hi, start, stop in MM_PLAN_BLOCK:
                        nc.tensor.matmul(
                            o_ps[:, m, lo:hi],
                            vtb[:, k, t : W : 4],
                            band_t[:, k, lo:hi],
                            start=start,
                            stop=stop,
                        )
                cb.copy(
                    o_sb[:, ci, 2 * half : 2 * half + 2, :],
                    o_ps[:NP_OUT],
                    2 * W,
                )
            if ci == G - 1:
                # one 16KB-per-partition store per 4-channel group,
                # dispatched from the otherwise-idle GpSimd (SWDGE)
                # queue (a HWDGE dispatch would block the sync/scalar
                # instruction stream on this group's copy dependencies,
                # starving input prefetch / later copies)
                nc.gpsimd.dma_start(out_ap[g], osbs.pop(g)[:])

        # software pipeline: PE runs pass1(c+1) while pass2(c) waits on
        # pass-1 copies
        pass1(0)
        for c in range(1, C):
            pass1(c)
            pass2(c - 1)
        pass2(C - 1)


def build_nc():
    import concourse.mybir as mybir
    import concourse.tile as tile
    from concourse import bacc

    # Bacc (not raw Bass): its compile() runs generate_event_semaphores,
    # which splits multi-semaphore waits — walrus codegen allows at most
    # one wait command per DMA instruction.
    nc = bacc.Bacc()
    x = nc.dram_tensor(
        "x", [C, P, NKH, W], mybir.dt.bfloat16, kind="ExternalInput"
    )
    band = nc.dram_tensor("band", [H, W], mybir.dt.bfloat16, kind="ExternalInput")
    out = nc.dram_tensor(
        "out", [NG, NP_OUT, G, 4, W], mybir.dt.bfloat16, kind="ExternalOutput"
    )
    with tile.TileContext(nc) as tc:
        build_avgpool(tc, x.ap(), band.ap(), out.ap())
    nc.compile()
    return nc


def _ensure_axon_ntff_hook():
    """If tracing is requested (BASS_TRACE) under axon, run_bass_kernel_spmd
    imports antenv.axon_hooks, which some agent images lack. Install the
    real hook if possible, else a stub that degrades tracing gracefully."""
    import sys
    import types

    try:
        import antenv.axon_hooks  # noqa: F401

        return
    except Exception:
        pass
    try:
        import antenv
    except Exception:
        return
    mod = types.ModuleType("antenv.axon_hooks")
    mod._hook = None
    mod.set_axon_ntff_profile_hook = lambda h: setattr(mod, "_hook", h)
    mod.get_axon_ntff_profile_hook = lambda: mod._hook
    sys.modules["antenv.axon_hooks"] = mod
    antenv.axon_hooks = mod
    try:
        from trn_agent_boot.trn_boot import _ntff_profile_via_ctypes

        hook = _ntff_profile_via_ctypes("/opt/axon/libaxon_pjrt.so")
        if hook is not None:
            mod.set_axon_ntff_profile_hook(hook)
    except Exception:
        pass


def prep_inputs(x: np.ndarray):
    """Shard, cast, and pre-block the full input for the 8 cores.

    Device x layout: x_dev[c, p, k, w] = x[c, 128k+p, w]
    """
    x = np.asarray(x, dtype=np.float32)
    assert x.shape == (8, C, H, W)
    xb = x.astype(ml_dtypes.bfloat16)
    band = make_band()
    in_maps = []
    for b in range(8):
        xd = np.ascontiguousarray(xb[b].reshape(C, NKH, P, W).transpose(0, 2, 1, 3))
        in_maps.append({"x": xd, "band": band})
    return in_maps


def gather_output(results) -> np.ndarray:
    """Unshard, un-block, upcast, and re-apply the replicate padding.

    Device out layout: out_dev[g, p, ci, t, w] = out[4g+ci, OLO+4p+t, w]
    """
    full = np.empty((8, C, H, W), dtype=np.float32)
    for b, r in enumerate(results):
        od = np.asarray(r["out"])  # [NG, NP_OUT, G, 4, W] bf16
        rows = od.transpose(0, 2, 1, 3, 4).reshape(C, NOUT, W).astype(np.float32)
        full[b, :, OLO:OHI] = rows
        full[b, :, :OLO] = rows[:, PT - OLO : PT - OLO + 1]
        full[b, :, OHI:] = rows[:, OUT_VALID + PT - 1 - OLO : OUT_VALID + PT - OLO]
    return full


def kernel(x) -> np.ndarray:
    _ensure_axon_ntff_hook()
    from concourse.bass_utils import run_bass_kernel_spmd

    nc = build_nc()
    in_maps = prep_inputs(x)
    res = run_bass_kernel_spmd(nc, in_maps, core_ids=list(range(8)))
    return gather_output(res.results)
